# revision 13
# baseline (speedup 1.0000x reference)
"""Causal self-attention (B=4, T=2048, C=1024, H=16) on 8 trn2 NeuronCores.

Sharding: core = (batch b, head-group hg) -> 4 x 2 grid. Each core computes
attention for 8 of the 16 heads of one batch plus the partial output
projection over its heads' columns; the host sums the two partials per batch
and adds b_proj.

Device-side layout choices (all matmul operands land in natural layout, no
on-device transposes):
  - host supplies x^T [C, T] and W^T slices in bf16
  - q,k are produced transposed [d, t] (lhsT = W^T chunk, rhs = x^T)
  - v is produced natural [t, d] (lhsT = x^T chunk, rhs = Wv^T)
  - scores are computed as S^T [tk, tq] (lhsT = k^T, rhs = q^T, K = d = 64,
    two heads packed into the PE array via partition offsets 0/64)
  - softmax skips max-subtraction (inputs are N(0,1)-scaled; exp stays finite)
  - softmax denominator comes from a ones-column appended to v (M = 65)
  - E@v accumulates y^T [d, tq] over key chunks in PSUM
  - causal masking: diagonal chunks get narrowed matmuls + one [128,128]
    triangular mask multiply
  - normalization: reciprocal of the denominator row, gpsimd
    partition_broadcast, fused multiply during the PSUM->SBUF copy
"""

import sys

if "/opt/trn_rl_repo" not in sys.path:
    sys.path.insert(0, "/opt/trn_rl_repo")

from contextlib import ExitStack

import ml_dtypes
import numpy as np

import concourse.bass as bass
import concourse.mybir as mybir
import concourse.tile as tile
from concourse import bacc
from concourse._compat import with_exitstack
from concourse.bass_utils import run_bass_kernel_spmd

BF16 = mybir.dt.bfloat16
F32 = mybir.dt.float32
NPBF16 = ml_dtypes.bfloat16

B, T, C, H = 4, 2048, 1024, 16
D = C // H              # 64
HPC = 8                 # heads per core
NPAIR = HPC // 2        # head pairs per core
NCORES = 8
CC = C // 128           # 8 contraction chunks
NQT = T // 512          # 4 query tiles of 512
NTT = T // 128          # 16 token tiles of 128
SCALE = 1.0 / np.sqrt(D)


@with_exitstack
def _attention_body(ctx: ExitStack, tc: tile.TileContext, t_in: dict, t_out):
    nc = tc.nc
    consts = ctx.enter_context(tc.tile_pool(name="consts", bufs=1))
    qkp = ctx.enter_context(tc.tile_pool(name="qkp", bufs=1))
    vp = ctx.enter_context(tc.tile_pool(name="vp", bufs=1))
    ytp = ctx.enter_context(tc.tile_pool(name="ytp", bufs=1))
    ep = ctx.enter_context(tc.tile_pool(name="ep", bufs=8))
    rp = ctx.enter_context(tc.tile_pool(name="rp", bufs=6))
    outp = ctx.enter_context(tc.tile_pool(name="outp", bufs=4))
    mm_ps = ctx.enter_context(tc.tile_pool(name="mm_ps", bufs=2, space="PSUM"))
    s_ps = ctx.enter_context(tc.tile_pool(name="s_ps", bufs=2, space="PSUM"))
    y_ps = ctx.enter_context(tc.tile_pool(name="y_ps", bufs=2, space="PSUM"))

    # ---- constants / inputs to SBUF ----
    xT = [consts.tile([128, T], BF16, tag=f"xT{c}", name=f"xT{c}") for c in range(CC)]
    wv = [consts.tile([128, 512], BF16, tag=f"wv{c}", name=f"wv{c}") for c in range(CC)]
    wqk = [consts.tile([128, 1024], BF16, tag=f"wqk{c}", name=f"wqk{c}") for c in range(CC)]
    wp = [consts.tile([128, 1024], BF16, tag=f"wp{j}", name=f"wp{j}") for j in range(NPAIR)]
    bqk = consts.tile([128, 8], F32, tag="bqk")
    nc.sync.dma_start(bqk[:], t_in["bqk"][:])
    bv_row = consts.tile([1, 512], F32, tag="bv_row")
    nc.sync.dma_start(bv_row[:], t_in["bv"][:])
    bv_bc = consts.tile([128, 512], F32, tag="bv_bc")
    nc.gpsimd.partition_broadcast(bv_bc[:], bv_row[:])
    tri = consts.tile([128, 128], BF16, tag="tri")
    nc.sync.dma_start(tri[:], t_in["tri"][:])
    # first 512 token columns of x arrive first: the first v-projection
    # groups (and slab-0 qk projections) unlock after ~2MB of DMA
    for c in range(CC):
        nc.sync.dma_start(xT[c][:, 0:512], t_in["xT"][c * 128:(c + 1) * 128, 0:512])
        nc.sync.dma_start(wv[c][:], t_in["wvT"][c * 128:(c + 1) * 128, :])
    for c in range(CC):
        nc.sync.dma_start(xT[c][:, 512:T], t_in["xT"][c * 128:(c + 1) * 128, 512:T])
    for c in range(CC):
        nc.sync.dma_start(wqk[c][:], t_in["wqkT"][c * 128:(c + 1) * 128, :])
    for j in range(NPAIR):
        nc.sync.dma_start(wp[j][:], t_in["wpT"][j * 128:(j + 1) * 128, :])

    # Projections, attention, and the output projection are interleaved per
    # 512-token slab: causal attention for q-tile qt only needs q/k/v of
    # slabs <= qt. Projection/output-projection matmul groups are spliced
    # between attention chunk-pairs so the PE has independent work while
    # ScalarE digests the exp backlog (ACT is the attention-phase
    # bottleneck otherwise).
    qk = [qkp.tile([128, T], BF16, tag=f"qk{j}", name=f"qk{j}") for j in range(CC)]
    v = [vp.tile([128, HPC, D + 1], BF16, tag=f"v{i}", name=f"v{i}") for i in range(NTT)]
    for i in range(NTT):
        nc.vector.memset(v[i][:, :, D:D + 1], 1.0)
    yt = [ytp.tile([128, T], BF16, tag=f"yt{j}", name=f"yt{j}") for j in range(NPAIR)]

    def vproj_group(i):
        ps = mm_ps.tile([128, 512], F32, tag="mm", name="ps_v")
        for cc in range(CC):
            nc.tensor.matmul(
                ps[:],
                xT[cc][:, i * 128:(i + 1) * 128],
                wv[cc][:],
                start=(cc == 0),
                stop=(cc == CC - 1),
            )
        nc.vector.tensor_add(
            v[i][:, :, 0:D],
            ps[:].rearrange("p (h d) -> p h d", h=HPC),
            bv_bc[:].rearrange("p (h d) -> p h d", h=HPC),
        )

    def qkproj_group(oc, tt):
        ps = mm_ps.tile([128, 512], F32, tag="mm", name="ps_qk")
        for cc in range(CC):
            nc.tensor.matmul(
                ps[:],
                wqk[cc][:, oc * 128:(oc + 1) * 128],
                xT[cc][:, tt * 512:(tt + 1) * 512],
                start=(cc == 0),
                stop=(cc == CC - 1),
            )
        nc.vector.tensor_scalar_add(
            qk[oc][:, tt * 512:(tt + 1) * 512], ps[:], bqk[:, oc:oc + 1]
        )

    def proj_slab(tt):
        # v projections are all emitted with slab 0 (they only need xT + wv,
        # which arrive first): they fill the PE while the wqk DMA completes.
        if tt == 0:
            for i in range(NTT):
                yield lambda i=i: vproj_group(i)
        for oc in range(CC):
            yield lambda oc=oc, tt=tt: qkproj_group(oc, tt)

    def outproj_group(i, oh):
        ps = mm_ps.tile([128, 512], F32, tag="mm", name="ps_op")
        for j in range(NPAIR):
            nc.tensor.matmul(
                ps[:],
                yt[j][:, i * 128:(i + 1) * 128],
                wp[j][:, oh * 512:(oh + 1) * 512],
                start=(j == 0),
                stop=(j == NPAIR - 1),
            )
        ob = obuf[i]
        nc.vector.tensor_copy(ob[:, oh * 512:(oh + 1) * 512], ps[:])
        if oh == 1:
            nc.sync.dma_start(t_out[i * 128:(i + 1) * 128, :], ob[:])

    obuf = {}

    def outproj_slab(qt):
        for i in range(4 * qt, 4 * qt + 4):
            obuf[i] = outp.tile([128, 1024], F32, tag="ob", name=f"ob{i}")
            for oh in range(2):
                yield lambda i=i, oh=oh: outproj_group(i, oh)

    def geom(qt, kc):
        m = kc - 4 * qt  # >= 0 on diagonal chunks
        qoff = 128 * m if m > 0 else 0
        return m, qoff, 512 - qoff

    def scores_chunk(qt, hp, kc):
        """Scores + exp + mask for one (head-pair, key-chunk); returns e."""
        q0 = qt * 512
        m, qoff, nw = geom(qt, kc)
        k0 = kc * 128
        sps = s_ps.tile([128, 1024], F32, tag="sps", name="sps")
        # head pair packed into the PE array at partition offsets 0/64
        nc.tensor.matmul(
            sps[:, 0:nw],
            qk[4 + hp][0:64, k0:k0 + 128],
            qk[hp][0:64, q0 + qoff:q0 + 512],
            start=True, stop=True,
        )
        nc.tensor.matmul(
            sps[:, 512:512 + nw],
            qk[4 + hp][64:128, k0:k0 + 128],
            qk[hp][64:128, q0 + qoff:q0 + 512],
            start=True, stop=True,
        )
        e = ep.tile([128, 1024], BF16, tag="e", name="e")
        nc.scalar.activation(
            e[:].rearrange("p (i n) -> p i n", i=2)[:, :, 0:nw],
            sps[:].rearrange("p (i n) -> p i n", i=2)[:, :, 0:nw],
            mybir.ActivationFunctionType.Exp,
            scale=float(SCALE),
        )
        if m >= 0:
            nc.vector.tensor_mul(
                e[:].rearrange("p (i n) -> p i n", i=2)[:, :, 0:128],
                e[:].rearrange("p (i n) -> p i n", i=2)[:, :, 0:128],
                tri[:].unsqueeze(1).broadcast_to([128, 2, 128]),
            )
        return e

    def ev_chunk(qt, hp, kc, ya, yb, e):
        kchunks = 4 * qt + 4
        _, qoff, nw = geom(qt, kc)
        nc.tensor.matmul(
            ya[:, qoff:512], v[kc][:, 2 * hp, :], e[:, 0:nw],
            start=(kc == 0), stop=(kc == kchunks - 1),
        )
        nc.tensor.matmul(
            yb[:, qoff:512], v[kc][:, 2 * hp + 1, :], e[:, 512:512 + nw],
            start=(kc == 0), stop=(kc == kchunks - 1),
        )

    def normalize(qt, hp, half, yps):
        # one fast copy evacuates the PSUM accumulator (freeing its bank for
        # the next head pair); the reciprocal/broadcast/multiply chain then
        # runs from SBUF off the PE-critical path.
        q0 = qt * 512
        ysb = rp.tile([D + 1, 512], F32, tag="ysb", name="ysb")
        nc.vector.tensor_copy(ysb[:], yps[:])
        rrow = rp.tile([1, 512], F32, tag="rrow", name="rrow")
        nc.vector.reciprocal(rrow[:], ysb[D:D + 1, :])
        rbc = rp.tile([64, 512], F32, tag="rbc", name="rbc")
        nc.gpsimd.partition_broadcast(rbc[:], rrow[:])
        nc.vector.tensor_mul(
            yt[hp][half * 64:half * 64 + 64, q0:q0 + 512],
            ysb[0:D, :],
            rbc[:],
        )

    def attn_slab(qt):
        # software-pipelined one chunk ahead: scores/exp of chunk kc+1 are
        # emitted before E@v of chunk kc, so the PE streams the next scores
        # while ScalarE computes the current exp.
        kchunks = 4 * qt + 4
        state = {}

        def step(hp, kc, ya, yb):
            if kc == 0:
                state["e"] = scores_chunk(qt, hp, 0)
            e = state["e"]
            if kc + 1 < kchunks:
                state["e"] = scores_chunk(qt, hp, kc + 1)
            ev_chunk(qt, hp, kc, ya, yb, e)

        for hp in range(NPAIR):
            ya = y_ps.tile([D + 1, 512], F32, tag="ya", name="ya", bufs=1)
            yb = y_ps.tile([D + 1, 512], F32, tag="yb", name="yb", bufs=1)
            for kc in range(kchunks):
                yield lambda hp=hp, kc=kc, ya=ya, yb=yb: step(hp, kc, ya, yb)
            yield lambda qt=qt, hp=hp, ya=ya, yb=yb: (
                normalize(qt, hp, 0, ya), normalize(qt, hp, 1, yb))

    def interleave(attn_work, fill_work):
        """Emit all of attn_work with fill_work spliced in evenly."""
        attn_work = list(attn_work)
        fill_work = list(fill_work)
        if not fill_work:
            for w in attn_work:
                w()
            return
        stride = max(1, len(attn_work) // (len(fill_work) + 1))
        fi = 0
        for n, w in enumerate(attn_work):
            w()
            if n % stride == stride - 1 and fi < len(fill_work):
                fill_work[fi]()
                fi += 1
        while fi < len(fill_work):
            fill_work[fi]()
            fi += 1

    # Fill schedule: attention of slab qt gets the next slab's projections;
    # the ScalarE-heaviest last slab gets all deferred output projections.
    for w in proj_slab(0):
        w()
    for qt in range(NQT):
        fill = []
        if qt < NQT - 1:
            fill.extend(proj_slab(qt + 1))
        else:
            for p in range(NQT - 1):
                fill.extend(outproj_slab(p))
        interleave(attn_slab(qt), fill)
    for w in outproj_slab(NQT - 1):
        w()


def build_model():
    nc = bacc.Bacc(
        "TRN2",
        target_bir_lowering=False,
        debug=False,
        enable_asserts=False,
        num_devices=NCORES,
    )
    t_in = {
        "xT": nc.dram_tensor("xT", [C, T], BF16, kind="ExternalInput").ap(),
        "wqkT": nc.dram_tensor("wqkT", [C, 1024], BF16, kind="ExternalInput").ap(),
        "wvT": nc.dram_tensor("wvT", [C, 512], BF16, kind="ExternalInput").ap(),
        "wpT": nc.dram_tensor("wpT", [512, C], BF16, kind="ExternalInput").ap(),
        "bqk": nc.dram_tensor("bqk", [128, 8], F32, kind="ExternalInput").ap(),
        "bv": nc.dram_tensor("bv", [1, 512], F32, kind="ExternalInput").ap(),
        "tri": nc.dram_tensor("tri", [128, 128], BF16, kind="ExternalInput").ap(),
    }
    t_out = nc.dram_tensor("out", [T, C], F32, kind="ExternalOutput").ap()
    with tile.TileContext(nc) as tc:
        _attention_body(tc, t_in, t_out)
    nc.compile()
    return nc


def make_in_maps(x, w_attn, b_attn, w_proj):
    """Host-side sharding: per-core input dict for core (b, hg)."""
    tri = np.triu(np.ones((128, 128), np.float32)).astype(NPBF16)
    in_maps = []
    xT_cache = {}
    for cid in range(NCORES):
        b, hg = cid // 2, cid % 2
        h0 = hg * HPC
        if b not in xT_cache:
            xT_cache[b] = np.ascontiguousarray(x[b].T).astype(NPBF16)
        rq = slice(h0 * D, (h0 + HPC) * D)
        rk = slice(C + h0 * D, C + (h0 + HPC) * D)
        rv = slice(2 * C + h0 * D, 2 * C + (h0 + HPC) * D)
        wqkT = np.ascontiguousarray(
            np.concatenate([w_attn[rq], w_attn[rk]], axis=0).T
        ).astype(NPBF16)
        wvT = np.ascontiguousarray(w_attn[rv].T).astype(NPBF16)
        wpT = np.ascontiguousarray(w_proj[:, h0 * D:(h0 + HPC) * D].T).astype(NPBF16)
        bqk = np.stack(
            [b_attn[rq].reshape(4, 128)[j] for j in range(4)]
            + [b_attn[rk].reshape(4, 128)[j] for j in range(4)],
            axis=1,
        ).astype(np.float32)
        bv = b_attn[rv].reshape(1, 512).astype(np.float32)
        in_maps.append({
            "xT": xT_cache[b],
            "wqkT": wqkT,
            "wvT": wvT,
            "wpT": wpT,
            "bqk": np.ascontiguousarray(bqk),
            "bv": bv,
            "tri": tri,
        })
    return in_maps


_NC_CACHE = []


def kernel(x, w_attn, b_attn, w_proj, b_proj):
    x = np.asarray(x, dtype=np.float32)
    w_attn = np.asarray(w_attn, dtype=np.float32)
    b_attn = np.asarray(b_attn, dtype=np.float32)
    w_proj = np.asarray(w_proj, dtype=np.float32)
    b_proj = np.asarray(b_proj, dtype=np.float32)

    if not _NC_CACHE:
        _NC_CACHE.append(build_model())
    nc = _NC_CACHE[0]
    in_maps = make_in_maps(x, w_attn, b_attn, w_proj)
    res = None
    for attempt in range(3):
        try:
            res = run_bass_kernel_spmd(nc, in_maps, core_ids=list(range(NCORES)))
            break
        except Exception:
            if attempt == 2:
                raise
            import time
            time.sleep(5)
    out = np.empty((B, T, C), np.float32)
    for b in range(B):
        out[b] = res.results[2 * b]["out"] + res.results[2 * b + 1]["out"]
    out += b_proj[None, None, :]
    return out


# revision 15
# speedup vs baseline: 1.0399x; 1.0399x over previous
"""Causal self-attention (B=4, T=2048, C=1024, H=16) on 8 trn2 NeuronCores.

Sharding: core = (batch b, head-group hg) -> 4 x 2 grid. Each core computes
attention for 8 of the 16 heads of one batch plus the partial output
projection over its heads' columns; the host sums the two partials per batch
and adds b_proj.

Device-side layout choices (all matmul operands land in natural layout, no
on-device transposes):
  - host supplies x^T [C, T] and W^T slices in bf16
  - q,k are produced transposed [d, t] (lhsT = W^T chunk, rhs = x^T)
  - v is produced natural [t, d] (lhsT = x^T chunk, rhs = Wv^T)
  - scores are computed as S^T [tk, tq] (lhsT = k^T, rhs = q^T, K = d = 64,
    two heads packed into the PE array via partition offsets 0/64)
  - softmax skips max-subtraction (inputs are N(0,1)-scaled; exp stays finite)
  - softmax denominator comes from a ones-column appended to v (M = 65)
  - E@v accumulates y^T [d, tq] over key chunks in PSUM
  - causal masking: diagonal chunks get narrowed matmuls + one [128,128]
    triangular mask multiply
  - normalization: reciprocal of the denominator row, gpsimd
    partition_broadcast, fused multiply during the PSUM->SBUF copy
"""

import sys

if "/opt/trn_rl_repo" not in sys.path:
    sys.path.insert(0, "/opt/trn_rl_repo")

from contextlib import ExitStack

import ml_dtypes
import numpy as np

import concourse.bass as bass
import concourse.mybir as mybir
import concourse.tile as tile
from concourse import bacc
from concourse._compat import with_exitstack
from concourse.bass_utils import run_bass_kernel_spmd

BF16 = mybir.dt.bfloat16
F32 = mybir.dt.float32
NPBF16 = ml_dtypes.bfloat16

B, T, C, H = 4, 2048, 1024, 16
D = C // H              # 64
HPC = 8                 # heads per core
NPAIR = HPC // 2        # head pairs per core
NCORES = 8
CC = C // 128           # 8 contraction chunks
NQT = T // 512          # 4 query tiles of 512
NTT = T // 128          # 16 token tiles of 128
SCALE = 1.0 / np.sqrt(D)


@with_exitstack
def _attention_body(ctx: ExitStack, tc: tile.TileContext, t_in: dict, t_out):
    nc = tc.nc
    consts = ctx.enter_context(tc.tile_pool(name="consts", bufs=1))
    qkp = ctx.enter_context(tc.tile_pool(name="qkp", bufs=1))
    vp = ctx.enter_context(tc.tile_pool(name="vp", bufs=1))
    ytp = ctx.enter_context(tc.tile_pool(name="ytp", bufs=1))
    ep = ctx.enter_context(tc.tile_pool(name="ep", bufs=8))
    rp = ctx.enter_context(tc.tile_pool(name="rp", bufs=6))
    outp = ctx.enter_context(tc.tile_pool(name="outp", bufs=4))
    mm_ps = ctx.enter_context(tc.tile_pool(name="mm_ps", bufs=2, space="PSUM"))
    s_ps = ctx.enter_context(tc.tile_pool(name="s_ps", bufs=2, space="PSUM"))
    y_ps = ctx.enter_context(tc.tile_pool(name="y_ps", bufs=2, space="PSUM"))

    # ---- constants / inputs to SBUF ----
    xT = [consts.tile([128, T], BF16, tag=f"xT{c}", name=f"xT{c}") for c in range(CC)]
    wv = [consts.tile([128, 512], BF16, tag=f"wv{c}", name=f"wv{c}") for c in range(CC)]
    wqk = [consts.tile([128, 1024], BF16, tag=f"wqk{c}", name=f"wqk{c}") for c in range(CC)]
    wp = [consts.tile([128, 1024], BF16, tag=f"wp{j}", name=f"wp{j}") for j in range(NPAIR)]
    bqk = consts.tile([128, 8], F32, tag="bqk")
    nc.sync.dma_start(bqk[:], t_in["bqk"][:])
    bv_row = consts.tile([1, 512], F32, tag="bv_row")
    nc.sync.dma_start(bv_row[:], t_in["bv"][:])
    bv_bc = consts.tile([128, 512], F32, tag="bv_bc")
    nc.gpsimd.partition_broadcast(bv_bc[:], bv_row[:])
    tri = consts.tile([128, 128], BF16, tag="tri")
    nc.sync.dma_start(tri[:], t_in["tri"][:])
    # progressive arrival: the first 512 token columns of x and wv land
    # first (unlocking the first v-projection groups within ~2MB of DMA);
    # the remaining x stripes alternate with wqk chunks so later v groups
    # unlock progressively while wqk streams in.
    for c in range(CC):
        nc.sync.dma_start(xT[c][:, 0:512], t_in["xT"][c * 128:(c + 1) * 128, 0:512])
        nc.sync.dma_start(wv[c][:], t_in["wvT"][c * 128:(c + 1) * 128, :])
    for s in range(3):
        lo, hi = 512 * (s + 1), 512 * (s + 2)
        for c in range(CC):
            nc.sync.dma_start(xT[c][:, lo:hi],
                              t_in["xT"][c * 128:(c + 1) * 128, lo:hi])
        for c in range(3 * s, 3 * s + (3 if s < 2 else 2)):
            nc.sync.dma_start(wqk[c][:], t_in["wqkT"][c * 128:(c + 1) * 128, :])
    for j in range(NPAIR):
        nc.sync.dma_start(wp[j][:], t_in["wpT"][j * 128:(j + 1) * 128, :])

    # Projections, attention, and the output projection are interleaved per
    # 512-token slab: causal attention for q-tile qt only needs q/k/v of
    # slabs <= qt. Projection/output-projection matmul groups are spliced
    # between attention chunk-pairs so the PE has independent work while
    # ScalarE digests the exp backlog (ACT is the attention-phase
    # bottleneck otherwise).
    qk = [qkp.tile([128, T], BF16, tag=f"qk{j}", name=f"qk{j}") for j in range(CC)]
    v = [vp.tile([128, HPC, D + 1], BF16, tag=f"v{i}", name=f"v{i}") for i in range(NTT)]
    for i in range(NTT):
        nc.vector.memset(v[i][:, :, D:D + 1], 1.0)
    yt = [ytp.tile([128, T], BF16, tag=f"yt{j}", name=f"yt{j}") for j in range(NPAIR)]

    def vproj_group(i):
        ps = mm_ps.tile([128, 512], F32, tag="mm", name="ps_v")
        for cc in range(CC):
            nc.tensor.matmul(
                ps[:],
                xT[cc][:, i * 128:(i + 1) * 128],
                wv[cc][:],
                start=(cc == 0),
                stop=(cc == CC - 1),
            )
        nc.vector.tensor_add(
            v[i][:, :, 0:D],
            ps[:].rearrange("p (h d) -> p h d", h=HPC),
            bv_bc[:].rearrange("p (h d) -> p h d", h=HPC),
        )

    def qkproj_group(oc, tt):
        ps = mm_ps.tile([128, 512], F32, tag="mm", name="ps_qk")
        for cc in range(CC):
            nc.tensor.matmul(
                ps[:],
                wqk[cc][:, oc * 128:(oc + 1) * 128],
                xT[cc][:, tt * 512:(tt + 1) * 512],
                start=(cc == 0),
                stop=(cc == CC - 1),
            )
        nc.vector.tensor_scalar_add(
            qk[oc][:, tt * 512:(tt + 1) * 512], ps[:], bqk[:, oc:oc + 1]
        )

    def proj_slab(tt):
        # v projections are all emitted with slab 0 (they only need xT + wv,
        # which arrive first): they fill the PE while the wqk DMA completes.
        if tt == 0:
            for i in range(NTT):
                yield lambda i=i: vproj_group(i)
        for oc in range(CC):
            yield lambda oc=oc, tt=tt: qkproj_group(oc, tt)

    def outproj_group(i, oh):
        ps = mm_ps.tile([128, 512], F32, tag="mm", name="ps_op")
        for j in range(NPAIR):
            nc.tensor.matmul(
                ps[:],
                yt[j][:, i * 128:(i + 1) * 128],
                wp[j][:, oh * 512:(oh + 1) * 512],
                start=(j == 0),
                stop=(j == NPAIR - 1),
            )
        ob = obuf[i]
        nc.vector.tensor_copy(ob[:, oh * 512:(oh + 1) * 512], ps[:])
        if i >= 4 * (NQT - 1):
            # final slab: ship each half as soon as its copy lands
            nc.sync.dma_start(
                t_out[i * 128:(i + 1) * 128, oh * 512:(oh + 1) * 512],
                ob[:, oh * 512:(oh + 1) * 512],
            )
        elif oh == 1:
            nc.sync.dma_start(t_out[i * 128:(i + 1) * 128, :], ob[:])

    obuf = {}

    def outproj_slab(qt):
        for i in range(4 * qt, 4 * qt + 4):
            obuf[i] = outp.tile([128, 1024], F32, tag="ob", name=f"ob{i}")
            for oh in range(2):
                yield lambda i=i, oh=oh: outproj_group(i, oh)

    def geom(qt, kc):
        m = kc - 4 * qt  # >= 0 on diagonal chunks
        qoff = 128 * m if m > 0 else 0
        return m, qoff, 512 - qoff

    def scores_chunk(qt, hp, kc):
        """Scores + exp + mask for one (head-pair, key-chunk); returns e."""
        q0 = qt * 512
        m, qoff, nw = geom(qt, kc)
        k0 = kc * 128
        sps = s_ps.tile([128, 1024], F32, tag="sps", name="sps")
        # head pair packed into the PE array at partition offsets 0/64
        nc.tensor.matmul(
            sps[:, 0:nw],
            qk[4 + hp][0:64, k0:k0 + 128],
            qk[hp][0:64, q0 + qoff:q0 + 512],
            start=True, stop=True,
        )
        nc.tensor.matmul(
            sps[:, 512:512 + nw],
            qk[4 + hp][64:128, k0:k0 + 128],
            qk[hp][64:128, q0 + qoff:q0 + 512],
            start=True, stop=True,
        )
        e = ep.tile([128, 1024], BF16, tag="e", name="e")
        nc.scalar.activation(
            e[:].rearrange("p (i n) -> p i n", i=2)[:, :, 0:nw],
            sps[:].rearrange("p (i n) -> p i n", i=2)[:, :, 0:nw],
            mybir.ActivationFunctionType.Exp,
            scale=float(SCALE),
        )
        if m >= 0:
            nc.vector.tensor_mul(
                e[:].rearrange("p (i n) -> p i n", i=2)[:, :, 0:128],
                e[:].rearrange("p (i n) -> p i n", i=2)[:, :, 0:128],
                tri[:].unsqueeze(1).broadcast_to([128, 2, 128]),
            )
        return e

    def ev_chunk(qt, hp, kc, ya, yb, e):
        kchunks = 4 * qt + 4
        _, qoff, nw = geom(qt, kc)
        nc.tensor.matmul(
            ya[:, qoff:512], v[kc][:, 2 * hp, :], e[:, 0:nw],
            start=(kc == 0), stop=(kc == kchunks - 1),
        )
        nc.tensor.matmul(
            yb[:, qoff:512], v[kc][:, 2 * hp + 1, :], e[:, 512:512 + nw],
            start=(kc == 0), stop=(kc == kchunks - 1),
        )

    def normalize(qt, hp, half, yps, evacuate=True):
        # one fast copy evacuates the PSUM accumulator (freeing its bank for
        # the next head pair); the reciprocal/broadcast/multiply chain then
        # runs from SBUF off the PE-critical path. The very last pair skips
        # the evacuation hop (nothing waits on its PSUM slot).
        q0 = qt * 512
        if evacuate:
            ysb = rp.tile([D + 1, 512], F32, tag="ysb", name="ysb")
            nc.vector.tensor_copy(ysb[:], yps[:])
        else:
            ysb = yps
        rrow = rp.tile([1, 512], F32, tag="rrow", name="rrow")
        nc.vector.reciprocal(rrow[:], ysb[D:D + 1, :])
        rbc = rp.tile([64, 512], F32, tag="rbc", name="rbc")
        nc.gpsimd.partition_broadcast(rbc[:], rrow[:])
        nc.vector.tensor_mul(
            yt[hp][half * 64:half * 64 + 64, q0:q0 + 512],
            ysb[0:D, :],
            rbc[:],
        )

    def attn_slab(qt):
        # software-pipelined one chunk ahead: scores/exp of chunk kc+1 are
        # emitted before E@v of chunk kc, so the PE streams the next scores
        # while ScalarE computes the current exp.
        kchunks = 4 * qt + 4
        state = {}

        def step(hp, kc, ya, yb):
            if kc == 0:
                state["e"] = scores_chunk(qt, hp, 0)
            e = state["e"]
            if kc + 1 < kchunks:
                state["e"] = scores_chunk(qt, hp, kc + 1)
            ev_chunk(qt, hp, kc, ya, yb, e)

        for hp in range(NPAIR):
            ya = y_ps.tile([D + 1, 512], F32, tag="ya", name="ya", bufs=1)
            yb = y_ps.tile([D + 1, 512], F32, tag="yb", name="yb", bufs=1)
            for kc in range(kchunks):
                yield lambda hp=hp, kc=kc, ya=ya, yb=yb: step(hp, kc, ya, yb)
            last = (qt == NQT - 1 and hp == NPAIR - 1)
            yield lambda qt=qt, hp=hp, ya=ya, yb=yb, ev=not last: (
                normalize(qt, hp, 0, ya, ev), normalize(qt, hp, 1, yb, ev))

    def interleave(attn_work, fill_work):
        """Emit all of attn_work with fill_work spliced in evenly."""
        attn_work = list(attn_work)
        fill_work = list(fill_work)
        if not fill_work:
            for w in attn_work:
                w()
            return
        stride = max(1, -(-len(attn_work) // (len(fill_work) + 1)))
        fi = 0
        for n, w in enumerate(attn_work):
            w()
            if n % stride == stride - 1 and fi < len(fill_work):
                fill_work[fi]()
                fi += 1
        while fi < len(fill_work):
            fill_work[fi]()
            fi += 1

    # Fill schedule: attention of slab qt gets the next slab's projections;
    # the ScalarE-heaviest last slab gets all deferred output projections.
    for w in proj_slab(0):
        w()
    for qt in range(NQT):
        fill = []
        if qt < NQT - 1:
            fill.extend(proj_slab(qt + 1))
        else:
            for p in range(NQT - 1):
                fill.extend(outproj_slab(p))
        interleave(attn_slab(qt), fill)
    for w in outproj_slab(NQT - 1):
        w()


def build_model():
    nc = bacc.Bacc(
        "TRN2",
        target_bir_lowering=False,
        debug=False,
        enable_asserts=False,
        num_devices=NCORES,
    )
    t_in = {
        "xT": nc.dram_tensor("xT", [C, T], BF16, kind="ExternalInput").ap(),
        "wqkT": nc.dram_tensor("wqkT", [C, 1024], BF16, kind="ExternalInput").ap(),
        "wvT": nc.dram_tensor("wvT", [C, 512], BF16, kind="ExternalInput").ap(),
        "wpT": nc.dram_tensor("wpT", [512, C], BF16, kind="ExternalInput").ap(),
        "bqk": nc.dram_tensor("bqk", [128, 8], F32, kind="ExternalInput").ap(),
        "bv": nc.dram_tensor("bv", [1, 512], F32, kind="ExternalInput").ap(),
        "tri": nc.dram_tensor("tri", [128, 128], BF16, kind="ExternalInput").ap(),
    }
    t_out = nc.dram_tensor("out", [T, C], F32, kind="ExternalOutput").ap()
    with tile.TileContext(nc) as tc:
        _attention_body(tc, t_in, t_out)
    nc.compile()
    return nc


def make_in_maps(x, w_attn, b_attn, w_proj):
    """Host-side sharding: per-core input dict for core (b, hg)."""
    tri = np.triu(np.ones((128, 128), np.float32)).astype(NPBF16)
    in_maps = []
    xT_cache = {}
    for cid in range(NCORES):
        b, hg = cid // 2, cid % 2
        h0 = hg * HPC
        if b not in xT_cache:
            xT_cache[b] = np.ascontiguousarray(x[b].T).astype(NPBF16)
        rq = slice(h0 * D, (h0 + HPC) * D)
        rk = slice(C + h0 * D, C + (h0 + HPC) * D)
        rv = slice(2 * C + h0 * D, 2 * C + (h0 + HPC) * D)
        wqkT = np.ascontiguousarray(
            np.concatenate([w_attn[rq], w_attn[rk]], axis=0).T
        ).astype(NPBF16)
        wvT = np.ascontiguousarray(w_attn[rv].T).astype(NPBF16)
        wpT = np.ascontiguousarray(w_proj[:, h0 * D:(h0 + HPC) * D].T).astype(NPBF16)
        bqk = np.stack(
            [b_attn[rq].reshape(4, 128)[j] for j in range(4)]
            + [b_attn[rk].reshape(4, 128)[j] for j in range(4)],
            axis=1,
        ).astype(np.float32)
        bv = b_attn[rv].reshape(1, 512).astype(np.float32)
        in_maps.append({
            "xT": xT_cache[b],
            "wqkT": wqkT,
            "wvT": wvT,
            "wpT": wpT,
            "bqk": np.ascontiguousarray(bqk),
            "bv": bv,
            "tri": tri,
        })
    return in_maps


_NC_CACHE = []


def kernel(x, w_attn, b_attn, w_proj, b_proj):
    x = np.asarray(x, dtype=np.float32)
    w_attn = np.asarray(w_attn, dtype=np.float32)
    b_attn = np.asarray(b_attn, dtype=np.float32)
    w_proj = np.asarray(w_proj, dtype=np.float32)
    b_proj = np.asarray(b_proj, dtype=np.float32)

    if not _NC_CACHE:
        _NC_CACHE.append(build_model())
    nc = _NC_CACHE[0]
    in_maps = make_in_maps(x, w_attn, b_attn, w_proj)
    res = None
    for attempt in range(3):
        try:
            res = run_bass_kernel_spmd(nc, in_maps, core_ids=list(range(NCORES)))
            break
        except Exception:
            if attempt == 2:
                raise
            import time
            time.sleep(5)
    out = np.empty((B, T, C), np.float32)
    for b in range(B):
        out[b] = res.results[2 * b]["out"] + res.results[2 * b + 1]["out"]
    out += b_proj[None, None, :]
    return out


# revision 22
# speedup vs baseline: 1.0423x; 1.0022x over previous
"""Causal self-attention (B=4, T=2048, C=1024, H=16) on 8 trn2 NeuronCores.

Sharding: core = (batch b, head-group hg) -> 4 x 2 grid. Each core computes
attention for 8 of the 16 heads of one batch plus the partial output
projection over its heads' columns; the host sums the two partials per batch
and adds b_proj.

Device-side layout choices (all matmul operands land in natural layout, no
on-device transposes):
  - host supplies x^T [C, T] and W^T slices in bf16
  - q,k are produced transposed [d, t] (lhsT = W^T chunk, rhs = x^T)
  - v is produced natural [t, d] (lhsT = x^T chunk, rhs = Wv^T)
  - scores are computed as S^T [tk, tq] (lhsT = k^T, rhs = q^T, K = d = 64,
    two heads packed into the PE array via partition offsets 0/64)
  - softmax skips max-subtraction (inputs are N(0,1)-scaled; exp stays finite)
  - softmax denominator comes from a ones-column appended to v (M = 65)
  - E@v accumulates y^T [d, tq] over key chunks in PSUM
  - causal masking: diagonal chunks get narrowed matmuls + one [128,128]
    triangular mask multiply
  - normalization: reciprocal of the denominator row, gpsimd
    partition_broadcast, fused multiply during the PSUM->SBUF copy
"""

import sys

if "/opt/trn_rl_repo" not in sys.path:
    sys.path.insert(0, "/opt/trn_rl_repo")

from contextlib import ExitStack

import ml_dtypes
import numpy as np

import concourse.bass as bass
import concourse.mybir as mybir
import concourse.tile as tile
from concourse import bacc
from concourse._compat import with_exitstack
from concourse.bass_utils import run_bass_kernel_spmd

BF16 = mybir.dt.bfloat16
F32 = mybir.dt.float32
NPBF16 = ml_dtypes.bfloat16

B, T, C, H = 4, 2048, 1024, 16
D = C // H              # 64
HPC = 8                 # heads per core
NPAIR = HPC // 2        # head pairs per core
NCORES = 8
CC = C // 128           # 8 contraction chunks
NQT = T // 512          # 4 query tiles of 512
NTT = T // 128          # 16 token tiles of 128
SCALE = 1.0 / np.sqrt(D)


@with_exitstack
def _attention_body(ctx: ExitStack, tc: tile.TileContext, t_in: dict, t_out):
    nc = tc.nc
    consts = ctx.enter_context(tc.tile_pool(name="consts", bufs=1))
    qkp = ctx.enter_context(tc.tile_pool(name="qkp", bufs=1))
    vp = ctx.enter_context(tc.tile_pool(name="vp", bufs=1))
    ytp = ctx.enter_context(tc.tile_pool(name="ytp", bufs=1))
    ep = ctx.enter_context(tc.tile_pool(name="ep", bufs=8))
    rp = ctx.enter_context(tc.tile_pool(name="rp", bufs=6))
    outp = ctx.enter_context(tc.tile_pool(name="outp", bufs=4))
    mm_ps = ctx.enter_context(tc.tile_pool(name="mm_ps", bufs=2, space="PSUM"))
    s_ps = ctx.enter_context(tc.tile_pool(name="s_ps", bufs=2, space="PSUM"))
    y_ps = ctx.enter_context(tc.tile_pool(name="y_ps", bufs=2, space="PSUM"))

    # ---- constants / inputs to SBUF ----
    xT = [consts.tile([128, T], BF16, tag=f"xT{c}", name=f"xT{c}") for c in range(CC)]
    wv = [consts.tile([128, 512], BF16, tag=f"wv{c}", name=f"wv{c}") for c in range(CC)]
    wqk = [consts.tile([128, 1024], BF16, tag=f"wqk{c}", name=f"wqk{c}") for c in range(CC)]
    wp = [consts.tile([128, 1024], BF16, tag=f"wp{j}", name=f"wp{j}") for j in range(NPAIR)]
    bqk = consts.tile([128, 8], F32, tag="bqk")
    nc.sync.dma_start(bqk[:], t_in["bqk"][:])
    bv_row = consts.tile([1, 512], F32, tag="bv_row")
    nc.sync.dma_start(bv_row[:], t_in["bv"][:])
    bv_bc = consts.tile([128, 512], F32, tag="bv_bc")
    nc.gpsimd.partition_broadcast(bv_bc[:], bv_row[:])
    tri = consts.tile([128, 128], BF16, tag="tri")
    nc.sync.dma_start(tri[:], t_in["tri"][:])
    # progressive arrival: the first 512 token columns of x and wv land
    # first (unlocking the first v-projection groups within ~2MB of DMA);
    # the remaining x stripes alternate with wqk chunks so later v groups
    # unlock progressively while wqk streams in.
    for c in range(CC):
        nc.sync.dma_start(xT[c][:, 0:512], t_in["xT"][c * 128:(c + 1) * 128, 0:512])
        nc.sync.dma_start(wv[c][:], t_in["wvT"][c * 128:(c + 1) * 128, :])
    for s in range(3):
        lo, hi = 512 * (s + 1), 512 * (s + 2)
        for c in range(CC):
            nc.sync.dma_start(xT[c][:, lo:hi],
                              t_in["xT"][c * 128:(c + 1) * 128, lo:hi])
        for c in range(3 * s, 3 * s + (3 if s < 2 else 2)):
            nc.sync.dma_start(wqk[c][:], t_in["wqkT"][c * 128:(c + 1) * 128, :])
    for j in range(NPAIR):
        nc.sync.dma_start(wp[j][:], t_in["wpT"][j * 128:(j + 1) * 128, :])

    # Projections, attention, and the output projection are interleaved per
    # 512-token slab: causal attention for q-tile qt only needs q/k/v of
    # slabs <= qt. Projection/output-projection matmul groups are spliced
    # between attention chunk-pairs so the PE has independent work while
    # ScalarE digests the exp backlog (ACT is the attention-phase
    # bottleneck otherwise).
    qk = [qkp.tile([128, T], BF16, tag=f"qk{j}", name=f"qk{j}") for j in range(CC)]
    v = [vp.tile([128, HPC, D + 1], BF16, tag=f"v{i}", name=f"v{i}") for i in range(NTT)]
    for i in range(NTT):
        nc.vector.memset(v[i][:, :, D:D + 1], 1.0)
    yt = [ytp.tile([128, T], BF16, tag=f"yt{j}", name=f"yt{j}") for j in range(NPAIR)]

    def vproj_group(i):
        ps = mm_ps.tile([128, 512], F32, tag="mm", name="ps_v")
        for cc in range(CC):
            nc.tensor.matmul(
                ps[:],
                xT[cc][:, i * 128:(i + 1) * 128],
                wv[cc][:],
                start=(cc == 0),
                stop=(cc == CC - 1),
            )
        nc.vector.tensor_add(
            v[i][:, :, 0:D],
            ps[:].rearrange("p (h d) -> p h d", h=HPC),
            bv_bc[:].rearrange("p (h d) -> p h d", h=HPC),
        )

    def qkproj_group(oc, tt):
        ps = mm_ps.tile([128, 512], F32, tag="mm", name="ps_qk")
        for cc in range(CC):
            nc.tensor.matmul(
                ps[:],
                wqk[cc][:, oc * 128:(oc + 1) * 128],
                xT[cc][:, tt * 512:(tt + 1) * 512],
                start=(cc == 0),
                stop=(cc == CC - 1),
            )
        nc.vector.tensor_scalar_add(
            qk[oc][:, tt * 512:(tt + 1) * 512], ps[:], bqk[:, oc:oc + 1]
        )

    def proj_slab(tt):
        # v projections are all emitted with slab 0 (they only need xT + wv,
        # which arrive first): they fill the PE while the wqk DMA completes.
        if tt == 0:
            for i in range(NTT):
                yield lambda i=i: vproj_group(i)
        for oc in range(CC):
            yield lambda oc=oc, tt=tt: qkproj_group(oc, tt)

    def outproj_group(i, oh):
        ps = mm_ps.tile([128, 512], F32, tag="mm", name="ps_op")
        for j in range(NPAIR):
            nc.tensor.matmul(
                ps[:],
                yt[j][:, i * 128:(i + 1) * 128],
                wp[j][:, oh * 512:(oh + 1) * 512],
                start=(j == 0),
                stop=(j == NPAIR - 1),
            )
        ob = obuf[i]
        nc.vector.tensor_copy(ob[:, oh * 512:(oh + 1) * 512], ps[:])
        if i >= 4 * (NQT - 1):
            # final slab: ship each half as soon as its copy lands
            nc.sync.dma_start(
                t_out[i * 128:(i + 1) * 128, oh * 512:(oh + 1) * 512],
                ob[:, oh * 512:(oh + 1) * 512],
            )
        elif oh == 1:
            nc.sync.dma_start(t_out[i * 128:(i + 1) * 128, :], ob[:])

    obuf = {}

    def outproj_slab(qt):
        for i in range(4 * qt, 4 * qt + 4):
            obuf[i] = outp.tile([128, 1024], F32, tag="ob", name=f"ob{i}")
            for oh in range(2):
                yield lambda i=i, oh=oh: outproj_group(i, oh)

    def geom(qt, kc):
        m = kc - 4 * qt  # >= 0 on diagonal chunks
        qoff = 128 * m if m > 0 else 0
        return m, qoff, 512 - qoff

    def scores_chunk(qt, hp, kc):
        """Scores + exp + mask for one (head-pair, key-chunk); returns e."""
        q0 = qt * 512
        m, qoff, nw = geom(qt, kc)
        k0 = kc * 128
        sps = s_ps.tile([128, 1024], F32, tag="sps", name="sps")
        # head pair packed into the PE array at partition offsets 0/64
        nc.tensor.matmul(
            sps[:, 0:nw],
            qk[4 + hp][0:64, k0:k0 + 128],
            qk[hp][0:64, q0 + qoff:q0 + 512],
            start=True, stop=True,
        )
        nc.tensor.matmul(
            sps[:, 512:512 + nw],
            qk[4 + hp][64:128, k0:k0 + 128],
            qk[hp][64:128, q0 + qoff:q0 + 512],
            start=True, stop=True,
        )
        e = ep.tile([128, 1024], BF16, tag="e", name="e")
        nc.scalar.activation(
            e[:].rearrange("p (i n) -> p i n", i=2)[:, :, 0:nw],
            sps[:].rearrange("p (i n) -> p i n", i=2)[:, :, 0:nw],
            mybir.ActivationFunctionType.Exp,
            scale=float(SCALE),
        )
        if m >= 0:
            nc.vector.tensor_mul(
                e[:].rearrange("p (i n) -> p i n", i=2)[:, :, 0:128],
                e[:].rearrange("p (i n) -> p i n", i=2)[:, :, 0:128],
                tri[:].unsqueeze(1).broadcast_to([128, 2, 128]),
            )
        return e

    def ev_chunk(qt, hp, kc, ya, yb, e):
        kchunks = 4 * qt + 4
        _, qoff, nw = geom(qt, kc)
        nc.tensor.matmul(
            ya[:, qoff:512], v[kc][:, 2 * hp, :], e[:, 0:nw],
            start=(kc == 0), stop=(kc == kchunks - 1),
        )
        nc.tensor.matmul(
            yb[:, qoff:512], v[kc][:, 2 * hp + 1, :], e[:, 512:512 + nw],
            start=(kc == 0), stop=(kc == kchunks - 1),
        )

    def normalize(qt, hp, half, yps, evacuate=True):
        # one fast copy evacuates the PSUM accumulator (freeing its bank for
        # the next head pair); the reciprocal/broadcast/multiply chain then
        # runs from SBUF off the PE-critical path. The very last pair skips
        # the evacuation hop (nothing waits on its PSUM slot).
        q0 = qt * 512
        if evacuate:
            ysb = rp.tile([D + 1, 512], F32, tag="ysb", name="ysb")
            nc.vector.tensor_copy(ysb[:], yps[:])
        else:
            ysb = yps
        rrow = rp.tile([1, 512], F32, tag="rrow", name="rrow")
        nc.vector.reciprocal(rrow[:], ysb[D:D + 1, :])
        rbc = rp.tile([64, 512], F32, tag="rbc", name="rbc")
        nc.gpsimd.partition_broadcast(rbc[:], rrow[:])
        nc.vector.tensor_mul(
            yt[hp][half * 64:half * 64 + 64, q0:q0 + 512],
            ysb[0:D, :],
            rbc[:],
        )

    def attn_slab(qt):
        # software-pipelined one chunk ahead ACROSS pair boundaries: the
        # scores/exp of the next (pair, chunk) are emitted before E@v of
        # the current one, so the PE always has the next scores to stream
        # while ScalarE computes the current exp.
        kchunks = 4 * qt + 4
        seq = [(hp, kc) for hp in range(NPAIR) for kc in range(kchunks)]
        state = {"y": None}

        def step(n):
            hp, kc = seq[n]
            if n == 0:
                state["e"] = scores_chunk(qt, hp, kc)
            e = state["e"]
            if n + 1 < len(seq):
                state["e"] = scores_chunk(qt, *seq[n + 1])
            if kc == 0:
                state["y"] = (
                    y_ps.tile([D + 1, 512], F32, tag="ya", name="ya", bufs=1),
                    y_ps.tile([D + 1, 512], F32, tag="yb", name="yb", bufs=1),
                )
            ev_chunk(qt, hp, kc, state["y"][0], state["y"][1], e)

        for n, (hp, kc) in enumerate(seq):
            yield lambda n=n: step(n)
            if kc == kchunks - 1:
                last = (qt == NQT - 1 and hp == NPAIR - 1)
                yield lambda qt=qt, hp=hp, y=state, ev=not last: (
                    normalize(qt, hp, 0, y["y"][0], ev),
                    normalize(qt, hp, 1, y["y"][1], ev))

    def interleave(attn_work, fill_work):
        """Emit all of attn_work with fill_work spliced in evenly."""
        attn_work = list(attn_work)
        fill_work = list(fill_work)
        if not fill_work:
            for w in attn_work:
                w()
            return
        stride = max(1, -(-len(attn_work) // (len(fill_work) + 1)))
        fi = 0
        for n, w in enumerate(attn_work):
            w()
            if n % stride == stride - 1 and fi < len(fill_work):
                fill_work[fi]()
                fi += 1
        while fi < len(fill_work):
            fill_work[fi]()
            fi += 1

    # Fill schedule: attention of slab qt gets the next slab's projections;
    # the ScalarE-heaviest last slab gets all deferred output projections.
    for w in proj_slab(0):
        w()
    for qt in range(NQT):
        fill = []
        if qt < NQT - 1:
            fill.extend(proj_slab(qt + 1))
        else:
            for p in range(NQT - 1):
                fill.extend(outproj_slab(p))
        interleave(attn_slab(qt), fill)
    for w in outproj_slab(NQT - 1):
        w()


def build_model():
    nc = bacc.Bacc(
        "TRN2",
        target_bir_lowering=False,
        debug=False,
        enable_asserts=False,
        num_devices=NCORES,
    )
    t_in = {
        "xT": nc.dram_tensor("xT", [C, T], BF16, kind="ExternalInput").ap(),
        "wqkT": nc.dram_tensor("wqkT", [C, 1024], BF16, kind="ExternalInput").ap(),
        "wvT": nc.dram_tensor("wvT", [C, 512], BF16, kind="ExternalInput").ap(),
        "wpT": nc.dram_tensor("wpT", [512, C], BF16, kind="ExternalInput").ap(),
        "bqk": nc.dram_tensor("bqk", [128, 8], F32, kind="ExternalInput").ap(),
        "bv": nc.dram_tensor("bv", [1, 512], F32, kind="ExternalInput").ap(),
        "tri": nc.dram_tensor("tri", [128, 128], BF16, kind="ExternalInput").ap(),
    }
    t_out = nc.dram_tensor("out", [T, C], F32, kind="ExternalOutput").ap()
    with tile.TileContext(nc) as tc:
        _attention_body(tc, t_in, t_out)
    nc.compile()
    return nc


def make_in_maps(x, w_attn, b_attn, w_proj):
    """Host-side sharding: per-core input dict for core (b, hg)."""
    tri = np.triu(np.ones((128, 128), np.float32)).astype(NPBF16)
    in_maps = []
    xT_cache = {}
    for cid in range(NCORES):
        b, hg = cid // 2, cid % 2
        h0 = hg * HPC
        if b not in xT_cache:
            xT_cache[b] = np.ascontiguousarray(x[b].T).astype(NPBF16)
        rq = slice(h0 * D, (h0 + HPC) * D)
        rk = slice(C + h0 * D, C + (h0 + HPC) * D)
        rv = slice(2 * C + h0 * D, 2 * C + (h0 + HPC) * D)
        wqkT = np.ascontiguousarray(
            np.concatenate([w_attn[rq], w_attn[rk]], axis=0).T
        ).astype(NPBF16)
        wvT = np.ascontiguousarray(w_attn[rv].T).astype(NPBF16)
        wpT = np.ascontiguousarray(w_proj[:, h0 * D:(h0 + HPC) * D].T).astype(NPBF16)
        bqk = np.stack(
            [b_attn[rq].reshape(4, 128)[j] for j in range(4)]
            + [b_attn[rk].reshape(4, 128)[j] for j in range(4)],
            axis=1,
        ).astype(np.float32)
        bv = b_attn[rv].reshape(1, 512).astype(np.float32)
        in_maps.append({
            "xT": xT_cache[b],
            "wqkT": wqkT,
            "wvT": wvT,
            "wpT": wpT,
            "bqk": np.ascontiguousarray(bqk),
            "bv": bv,
            "tri": tri,
        })
    return in_maps


_NC_CACHE = []


def kernel(x, w_attn, b_attn, w_proj, b_proj):
    x = np.asarray(x, dtype=np.float32)
    w_attn = np.asarray(w_attn, dtype=np.float32)
    b_attn = np.asarray(b_attn, dtype=np.float32)
    w_proj = np.asarray(w_proj, dtype=np.float32)
    b_proj = np.asarray(b_proj, dtype=np.float32)

    if not _NC_CACHE:
        _NC_CACHE.append(build_model())
    nc = _NC_CACHE[0]
    in_maps = make_in_maps(x, w_attn, b_attn, w_proj)
    res = None
    for attempt in range(3):
        try:
            res = run_bass_kernel_spmd(nc, in_maps, core_ids=list(range(NCORES)))
            break
        except Exception:
            if attempt == 2:
                raise
            import time
            time.sleep(5)
    out = np.empty((B, T, C), np.float32)
    for b in range(B):
        out[b] = res.results[2 * b]["out"] + res.results[2 * b + 1]["out"]
    out += b_proj[None, None, :]
    return out
